# revision 4
# baseline (speedup 1.0000x reference)
"""KAN layer (B=8192, in=128, out=128, cubic B-spline, 16-knot grid) on 8 trn2 cores.

Math: out[b,o] = mean_i tanh(x)[b,i] + (1/128) sum_{i,k} basis[b,i,k] cf[o,i,k]
with basis_k(s) = B3(s - k), s = 7.5*tanh(x) + 10.5 in (3, 18), and B3 the
cardinal cubic B-spline. Using the exact truncated-power identity
    B3(u) = (1/6) * sum_{r=0}^{4} (-1)^r C(4,r) relu(u - r)^3
the spline term becomes sum_{i,j} relu(s - j)^3 * cf2[o,i,j] (j = 0..21) where
cf2 is cf convolved with [1,-4,6,-4,1]/768 along k — a dense fp32 matmul over
(i, j) against 22 "plane" tensors relu(s-j)^3 [i, b], each built by ONE fused
custom-DVE op. The mean-residual term is matmul block 0 (weights = 1/128, rhs
= tanh(x)). Sharding: pure data-parallel over batch; weights replicated.
"""

import os
from contextlib import ExitStack

import numpy as np

os.environ.setdefault("MYCRO_LOCAL_CACHE", "1")

import concourse.bacc as bacc
import concourse.bass as bass  # noqa: F401
import concourse.mybir as mybir
import concourse.tile as tile
from concourse import bass_utils
import concourse.dve_ops as dve_ops
from concourse.dve_ops import DveOp
from concourse.dve_spec import Spec, Src0, C0, relu, sq, lower, _has_src1
from concourse.dve_uop import DveOpSpec

F32 = mybir.dt.float32
N_CORES = 8
B = 8192
IN = 128
OUT = 128
NB = 18          # spline basis functions per (o, i)
NJ = NB + 4      # truncated-power planes
BC = B // N_CORES  # batch per core = 1024
HALF = BC // 2     # fp32 matmul moving-operand limit = 512


def _register_op(name, spec, subdim=False):
    for op in dve_ops.OPS:
        if op.name == name:
            return op
    row = dve_ops._CUSTOM_DVE_ROW_BASE + len(dve_ops.OPS)
    assert row < 0x20, "custom DVE opcode rows exhausted"
    sha = {}
    for ver in ("v3",):
        tmp = DveOpSpec(name=name, opcode=row, uops=lower(spec, ver=ver),
                        rd1_en=_has_src1(spec))
        sha[ver] = tmp.sha(ver)
    op = DveOp(name, spec, subdim=subdim, uops_sha=sha)
    dve_ops.OPS.append(op)
    dve_ops.CUSTOM_DVE_SPECS[name] = spec
    dve_ops._SUB_OPCODE_FOR_NAME[name] = row
    return op


def _rcube_op():
    # out = relu(in0 - s0)^3   (4 ALU stages)
    r = relu(Src0 - C0)
    spec = Spec(
        body=sq(r) * r,
        reference=lambda in0, in1, s0, s1, imm2: np.maximum(
            in0.astype(np.float32) - s0, 0.0) ** 3,
    )
    return _register_op("RCUBE_ANT", spec)


def _weights(coef: np.ndarray) -> np.ndarray:
    """[IN, (1+NJ)*OUT] fp32: block 0 = 1/128 (mean residual, rhs=tanh(x));
    block j+1 [i, o] = cf2[o, i, j]."""
    cf = coef.reshape(OUT, IN, NB).astype(np.float64)
    binom = np.array([1.0, -4.0, 6.0, -4.0, 1.0]) / (6.0 * 128.0)
    cf2 = np.zeros((OUT, IN, NJ))
    for r in range(5):
        cf2[:, :, r:r + NB] += binom[r] * cf
    wts = np.empty((IN, (1 + NJ) * OUT), np.float32)
    wts[:, :OUT] = 1.0 / IN
    wts[:, OUT:] = cf2.transpose(1, 2, 0).reshape(IN, NJ * OUT)
    return wts


_NC = {}


def _build(reps: int = 1):
    if reps in _NC:
        return _NC[reps]
    rcube = _rcube_op()
    nc = bacc.Bacc("TRN2", target_bir_lowering=False, debug=False,
                   enable_asserts=False, num_devices=N_CORES)
    xT = nc.dram_tensor("xT", [IN, BC], F32, kind="ExternalInput").ap()
    wts = nc.dram_tensor("wts", [IN, (1 + NJ) * OUT], F32,
                         kind="ExternalInput").ap()
    outT = nc.dram_tensor("outT", [OUT, BC], F32, kind="ExternalOutput").ap()

    with tile.TileContext(nc) as tc:
        with ExitStack() as ctx:
            iop = ctx.enter_context(tc.tile_pool(name="io", bufs=1))
            planes = ctx.enter_context(tc.tile_pool(name="planes", bufs=6))
            psp = ctx.enter_context(tc.tile_pool(name="ps", bufs=1, space="PSUM"))

            w = iop.tile([IN, (1 + NJ) * OUT], F32, tag="w")
            nc.sync.dma_start(w[:], wts[:])
            for _rep in range(reps):
                xs = iop.tile([IN, BC], F32, tag="xs")
                nc.sync.dma_start(xs[:], xT[:])

                xt = iop.tile([IN, BC], F32, tag="xt")
                nc.scalar.activation(xt[:], xs[:],
                                     mybir.ActivationFunctionType.Tanh)
                s = iop.tile([IN, BC], F32, tag="s")
                nc.vector.tensor_scalar(s[:], xt[:], 7.5, 10.5,
                                        mybir.AluOpType.mult,
                                        mybir.AluOpType.add)

                ps0 = psp.tile([OUT, HALF], F32, tag="ps0")
                ps1 = psp.tile([OUT, HALF], F32, tag="ps1")
                ps = [ps0, ps1]
                for t in range(2):
                    nc.tensor.matmul(ps[t][:], w[:, 0:OUT],
                                     xt[:, t * HALF:(t + 1) * HALF],
                                     start=True, stop=False)
                for j in range(NJ):
                    pl = planes.tile([IN, BC], F32, tag="plane")
                    nc.vector._custom_dve(rcube, out=pl[:], in0=s[:],
                                          s0=float(j))
                    wblk = w[:, (j + 1) * OUT:(j + 2) * OUT]
                    for t in range(2):
                        nc.tensor.matmul(ps[t][:], wblk,
                                         pl[:, t * HALF:(t + 1) * HALF],
                                         start=False, stop=(j == NJ - 1))

                osb = iop.tile([OUT, BC], F32, tag="osb")
                for t in range(2):
                    nc.scalar.copy(osb[:, t * HALF:(t + 1) * HALF], ps[t][:])
                nc.sync.dma_start(outT[:], osb[:])

    nc.compile()
    _NC[reps] = nc
    return nc


def run(x: np.ndarray, coef: np.ndarray, trace: bool = False, reps: int = 1):
    nc = _build(reps)
    x = np.ascontiguousarray(np.asarray(x, dtype=np.float32))
    coef = np.asarray(coef, dtype=np.float32)
    wts = _weights(coef)
    in_maps = []
    for c in range(N_CORES):
        xc = np.ascontiguousarray(x[c * BC:(c + 1) * BC, :].T)
        in_maps.append({"xT": xc, "wts": wts})
    res = bass_utils.run_bass_kernel_spmd(
        nc, in_maps, core_ids=list(range(N_CORES)), trace=trace)
    full = np.concatenate([r["outT"] for r in res.results], axis=1)  # [OUT, B]
    out = np.ascontiguousarray(full.T)  # [B, OUT]
    return out, res


def kernel(x: np.ndarray, coef: np.ndarray) -> np.ndarray:
    out, _ = run(x, coef, trace=False)
    return out


if __name__ == "__main__":
    rng = np.random.RandomState(0)
    x = rng.randn(B, IN).astype(np.float32)
    coef = (0.5 * rng.randn(OUT * IN, NB)).astype(np.float32)
    out = kernel(x, coef)
    print("out", out.shape, out.dtype, float(np.abs(out).max()))
